# revision 28
# baseline (speedup 1.0000x reference)
"""DepthPolarReducer Trainium2 kernel.

Full-input contract: kernel(**inputs) takes the complete arrays and returns the
complete (64, 32) float32 output. The batch is sharded 8 ways across the 8
NeuronCores (pure data parallel, bin_weights replicated, no collectives).

Math (identical to the reference up to f32 rounding):
    dm  = dc*mc + (1-mc)*100                      (cropped rows 192:480)
    out[b, nb] = -log( sum_w (w[nb,w]+1e-10) * sum_h mc*exp(-20*dm) ) / 20
The reference's two-level stable logsumexp collapses algebraically to this
unnormalized form: any term more than ~e^-87 below a bin's dominant column
underflows to 0 in f32 in the reference as well, so results agree to ~1e-4
for any random-fill data (verified against the oracle).

Host prep (cheap affine/masking/layout only — all transcendentals and
reductions run on device):
  - crop rows, shard batch, subtract 100, apply binary mask (masked -> 0,
    so exp(-20*0 - 2000) == 0 on device), and lay rows out so each DMA is
    per-partition contiguous and every 128-row tile holds 16 rows of each
    of the core's 8 images (-> constant one-hot PE reduction matrix).

Device schedule per core (fast path, binary mask):
    6 chunks of [128, 3*640]:
        ACT: e = exp(-20*v - 2000)        (v = masked depth-100 from host)
        PE:  col0[img, :] += onehot.T @ e   (f32r single-pass, PSUM accum)
    tail: col0 [8, 640] -> PE transpose -> [640, 8] -> PE @ w_t -> S [8, 32]
          ACT: log(S + 1e-30); DVE: * -1/20; DMA out.
General path (continuous mask) adds a mask tensor and one DVE multiply
(e = e * m) before the PE accumulation.
"""

import numpy as np

import concourse.bass as bass
import concourse.tile as tile
from concourse import bacc, mybir

# ---------------------------------------------------------------- constants
N_CORES = 8
BATCH = 64
H_IMG = 480
W_IMG = 640
CROP_START = 192
CROP_H = H_IMG - CROP_START          # 288
NUM_BINS = 32
KAPPA = 20.0

B_PER = BATCH // N_CORES             # 8 images per core
ROWS = B_PER * CROP_H                # 2304
P = 128
NT = ROWS // P                       # 18 row tiles
RPT = P // B_PER                     # 16 rows of each image per tile
WJ = W_IMG // P                      # 5 column chunks of 128

F32 = mybir.dt.float32
F32R = mybir.dt.float32r
U8 = mybir.dt.uint8

USE_F32R = True                      # single-pass PE col-sums (~7e-5 rel err)
# tapered chunk sizes (in 128-row tiles): small ends shorten pipeline
# fill/drain, middle chunks amortize DMA trigger cost
CHUNKS = (1, 1, 1, 1, 1, 1, 2, 2, 2, 2, 2, 2)
assert sum(CHUNKS) == NT
FREE = NT * W_IMG                    # 11520 free elements per partition


def _build_nc(binary_mask: bool) -> bass.Bass:
    nc = bacc.Bacc(trn_type="TRN2")

    e_dt = F32R if USE_F32R else F32

    depth = nc.dram_tensor("depth", [P, FREE], F32, kind="ExternalInput")
    if not binary_mask:
        maskv = nc.dram_tensor("maskv", [P, FREE], F32, kind="ExternalInput")
    w_t = nc.dram_tensor("w_t", [WJ, P, NUM_BINS], F32, kind="ExternalInput")
    onehot = nc.dram_tensor("onehot", [P, B_PER], F32, kind="ExternalInput")
    ident8 = nc.dram_tensor("ident8", [B_PER, B_PER], F32, kind="ExternalInput")
    out = nc.dram_tensor("out", [B_PER, NUM_BINS], F32, kind="ExternalOutput")

    with tile.TileContext(nc) as tc:
        with (
            tc.tile_pool(name="consts", bufs=1) as consts,
            tc.tile_pool(name="data", bufs=8) as data,
            tc.tile_pool(name="work", bufs=8) as work,
            tc.tile_pool(name="tail", bufs=1) as tail,
            tc.tile_pool(name="psum", bufs=1, space="PSUM") as psum,
        ):
            # constants. oh_buf MUST be the first SBUF allocation and padded
            # to a full 512B row: the f32r LDWEIGHTS path corrupts weight
            # columns 4-7 when the one-hot sits at other offsets/pitches.
            oh_buf = consts.tile([P, P], e_dt)
            oh_sb = oh_buf[:, 0:B_PER]
            nc.scalar.dma_start(
                out=oh_sb,
                in_=onehot[:, :].bitcast(e_dt) if USE_F32R else onehot[:, :],
            )
            wt_sb = consts.tile([P, WJ, NUM_BINS], F32)
            nc.scalar.dma_start(out=wt_sb, in_=w_t.rearrange("j p n -> p j n"))
            id_sb = consts.tile([B_PER, B_PER], F32)
            nc.scalar.dma_start(out=id_sb, in_=ident8[:, :])
            bias_exp = consts.tile([P, 1], F32)
            nc.vector.memset(bias_exp, -KAPPA * 100.0)
            bias_ln = consts.tile([P, 1], F32)
            nc.vector.memset(bias_ln, 1e-30)
            # dummy activation so the Exp table load is hoisted to kernel
            # start, overlapping the first data DMA instead of blocking it
            warm = consts.tile([1, 1], F32)
            nc.scalar.activation(
                warm, bias_ln[0:1, :], mybir.ActivationFunctionType.Exp,
                bias=bias_exp[0:1, :], scale=0.0,
            )

            # stage 1: per-image column sums, accumulated in PSUM.
            col_a = psum.tile([B_PER, 512], F32)
            col_b = psum.tile([B_PER, W_IMG - 512], F32)
            t0 = 0
            for c, ch in enumerate(CHUNKS):
                cw = ch * W_IMG
                off = t0 * W_IMG
                dma_eng = nc.sync
                d_t = data.tile([P, ch, W_IMG], F32, tag=f"d{ch}")
                dma_eng.dma_start(
                    out=d_t,
                    in_=depth[:, off : off + cw].rearrange(
                        "p (n w) -> p n w", w=W_IMG
                    ),
                )
                if binary_mask:
                    e_t = work.tile([P, ch, W_IMG], e_dt, tag=f"e{ch}")
                    nc.scalar.activation(
                        e_t, d_t, mybir.ActivationFunctionType.Exp,
                        bias=bias_exp, scale=-KAPPA,
                    )
                else:
                    m_t = data.tile([P, ch, W_IMG], F32, tag=f"m{ch}")
                    dma_eng.dma_start(
                        out=m_t,
                        in_=maskv[:, off : off + cw].rearrange(
                            "p (n w) -> p n w", w=W_IMG
                        ),
                    )
                    e_f = work.tile([P, ch, W_IMG], F32, tag=f"ef{ch}")
                    nc.scalar.activation(
                        e_f, d_t, mybir.ActivationFunctionType.Exp,
                        bias=bias_exp, scale=-KAPPA,
                    )
                    e_t = work.tile([P, ch, W_IMG], e_dt, tag=f"e{ch}")
                    nc.vector.tensor_mul(e_t, e_f, m_t)

                for n in range(ch):
                    first = t0 + n == 0
                    last = t0 + n == NT - 1
                    nc.tensor.matmul(
                        col_a, oh_sb, e_t[:, n, 0:512],
                        start=first, stop=last,
                    )
                    nc.tensor.matmul(
                        col_b, oh_sb, e_t[:, n, 512:W_IMG],
                        start=first, stop=last,
                    )
                t0 += ch

            # stage 2: S[img, nb] = sum_w col0[img, w] * w_t[w, nb]
            col_sb = tail.tile([B_PER, W_IMG], F32)
            nc.vector.tensor_copy(col_sb[:, 0:512], col_a)
            nc.scalar.copy(col_sb[:, 512:W_IMG], col_b)

            colT_ps = psum.tile([P, WJ, B_PER], F32)
            for j in range(WJ):
                nc.tensor.transpose(
                    colT_ps[:, j, :],
                    col_sb[:, j * P : (j + 1) * P],
                    id_sb,
                )
            colT_sb = tail.tile([P, WJ, B_PER], F32)
            nc.vector.tensor_copy(colT_sb, colT_ps)

            s_ps = psum.tile([B_PER, NUM_BINS], F32)
            for j in range(WJ):
                nc.tensor.matmul(
                    s_ps, colT_sb[:, j, :], wt_sb[:, j, :],
                    start=(j == 0), stop=(j == WJ - 1),
                )

            res = tail.tile([B_PER, NUM_BINS], F32)
            nc.scalar.activation(
                res, s_ps, mybir.ActivationFunctionType.Ln,
                bias=bias_ln[:B_PER], scale=1.0,
            )
            nc.vector.tensor_scalar_mul(res, res, -1.0 / KAPPA)
            nc.sync.dma_start(out=out[:, :], in_=res)

    nc.compile()
    return nc


# ------------------------------------------------------------------ runner
_RUNNERS: dict = {}


def _make_runner(binary_mask: bool):
    """Build the Bass module once and return a persistent jitted SPMD runner.

    Mirrors concourse.bass2jax.run_bass_via_pjrt's multi-core path, but keeps
    the jax.jit object alive across calls so repeat invocations skip
    retracing/recompilation.
    """
    import jax
    from jax.experimental.shard_map import shard_map
    from jax.sharding import Mesh, PartitionSpec

    from concourse import bass2jax, mybir as _mybir

    nc = _build_nc(binary_mask)
    bass2jax.install_neuronx_cc_hook()

    partition_name = (
        nc.partition_id_tensor.name if nc.partition_id_tensor else None
    )
    in_names, out_names, out_avals, zero_outs = [], [], [], []
    for alloc in nc.m.functions[0].allocations:
        if not isinstance(alloc, _mybir.MemoryLocationSet):
            continue
        name = alloc.memorylocations[0].name
        if alloc.kind == "ExternalInput":
            if name != partition_name:
                in_names.append(name)
        elif alloc.kind == "ExternalOutput":
            shape = tuple(alloc.tensor_shape)
            dtype = _mybir.dt.np(alloc.dtype)
            out_names.append(name)
            out_avals.append(jax.core.ShapedArray(shape, dtype))
            zero_outs.append(np.zeros(shape, dtype))
    n_params = len(in_names)
    n_outs = len(out_avals)
    all_in_names = list(in_names) + list(out_names)
    if partition_name is not None:
        all_in_names.append(partition_name)

    def _body(*args):
        operands = list(args)
        if partition_name is not None:
            operands.append(bass2jax.partition_id_tensor())
        outs = bass2jax._bass_exec_p.bind(
            *operands,
            out_avals=tuple(out_avals),
            in_names=tuple(all_in_names),
            out_names=tuple(out_names),
            lowering_input_output_aliases=(),
            sim_require_finite=True,
            sim_require_nnan=True,
            nc=nc,
        )
        return tuple(outs)

    devices = jax.devices()[:N_CORES]
    assert len(devices) == N_CORES, f"need {N_CORES} cores, have {len(devices)}"
    mesh = Mesh(np.asarray(devices), ("core",))
    donate = tuple(range(n_params, n_params + n_outs))
    sharded = jax.jit(
        shard_map(
            _body,
            mesh=mesh,
            in_specs=(PartitionSpec("core"),) * (n_params + n_outs),
            out_specs=(PartitionSpec("core"),) * n_outs,
            check_rep=False,
        ),
        donate_argnums=donate,
        keep_unused=True,
    )

    def run(per_core_in_maps):
        concat_in = [
            np.concatenate([m[name] for m in per_core_in_maps], axis=0)
            for name in in_names
        ]
        concat_zeros = [
            np.zeros((N_CORES * z.shape[0], *z.shape[1:]), z.dtype)
            for z in zero_outs
        ]
        out_arrs = sharded(*concat_in, *concat_zeros)
        return [
            {
                name: np.asarray(out_arrs[i]).reshape(
                    N_CORES, *out_avals[i].shape
                )[c]
                for i, name in enumerate(out_names)
            }
            for c in range(N_CORES)
        ]

    return run


def _get_runner(binary_mask: bool):
    key = "u8" if binary_mask else "f32"
    if key not in _RUNNERS:
        _RUNNERS[key] = _make_runner(binary_mask)
    return _RUNNERS[key]


_ONEHOT = np.zeros((P, B_PER), np.float32)
_ONEHOT[np.arange(P), np.arange(P) // RPT] = 1.0
_IDENT8 = np.eye(B_PER, dtype=np.float32)


def _to_device_layout(a):
    """[N_CORES, B_PER, CROP_H, W] -> [N_CORES, 128, NT*W] so that partition
    p = img*16 + r16 reads its rows (r16::16) contiguously."""
    a = a.reshape(N_CORES, B_PER, NT, RPT, W_IMG)
    a = a.transpose(0, 1, 3, 2, 4)                  # [., img, r16, t, w]
    return np.ascontiguousarray(a.reshape(N_CORES, P, FREE))


def _prep_in_maps(inputs):
    depth_map = np.asarray(inputs["depth_map"], np.float32)
    depth_mask = np.asarray(inputs["depth_mask"], np.float32)
    bin_weights = np.asarray(inputs["bin_weights"], np.float32)

    dc = depth_map[:, CROP_START:, :].reshape(N_CORES, B_PER, CROP_H, W_IMG)
    mcf = depth_mask[:, CROP_START:, :].reshape(N_CORES, B_PER, CROP_H, W_IMG)
    binary = bool(np.all((mcf == 0.0) | (mcf == 1.0)))

    if binary:
        v = np.where(mcf != 0.0, dc - np.float32(100.0), np.float32(0.0))
        v = _to_device_layout(v)
        masks = None
    else:
        v = _to_device_layout(dc - np.float32(100.0))
        masks = _to_device_layout(mcf)

    w_t = np.ascontiguousarray(
        (bin_weights.astype(np.float64) + 1e-10).astype(np.float32).T
    ).reshape(WJ, P, NUM_BINS)

    in_maps = []
    for c in range(N_CORES):
        m = {
            "depth": v[c],
            "w_t": w_t,
            "onehot": _ONEHOT,
            "ident8": _IDENT8,
        }
        if masks is not None:
            m["maskv"] = masks[c]
        in_maps.append(m)
    return in_maps, binary


def kernel(**inputs) -> np.ndarray:
    bin_weights = np.asarray(inputs["bin_weights"], np.float32)
    in_maps, binary = _prep_in_maps(inputs)
    run = _get_runner(binary)
    results = run(in_maps)
    out = np.concatenate([results[c]["out"] for c in range(N_CORES)], axis=0)

    w_sum = bin_weights.sum(axis=-1) * CROP_H
    return np.where(w_sum[None, :] < 1e-6, np.float32(100.0), out).astype(
        np.float32
    )


# revision 30
# speedup vs baseline: 1.0805x; 1.0805x over previous
"""DepthPolarReducer Trainium2 kernel.

Full-input contract: kernel(**inputs) takes the complete arrays and returns the
complete (64, 32) float32 output. The batch is sharded 8 ways across the 8
NeuronCores (pure data parallel, bin_weights replicated, no collectives).

Math (identical to the reference up to f32 rounding):
    dm  = dc*mc + (1-mc)*100                      (cropped rows 192:480)
    out[b, nb] = -log( sum_w (w[nb,w]+1e-10) * sum_h mc*exp(-20*dm) ) / 20
The reference's two-level stable logsumexp collapses algebraically to this
unnormalized form: any term more than ~e^-87 below a bin's dominant column
underflows to 0 in f32 in the reference as well, so results agree to ~1e-4
for any random-fill data (verified against the oracle).

Host prep (cheap affine/masking/layout only — all transcendentals and
reductions run on device):
  - crop rows, shard batch, subtract 100, apply binary mask (masked -> 0,
    so exp(-20*0 - 2000) == 0 on device), and lay rows out so each DMA is
    per-partition contiguous and every 128-row tile holds 16 rows of each
    of the core's 8 images (-> constant one-hot PE reduction matrix).

Device schedule per core (fast path, binary mask):
    6 chunks of [128, 3*640]:
        ACT: e = exp(-20*v - 2000)        (v = masked depth-100 from host)
        PE:  col0[img, :] += onehot.T @ e   (f32r single-pass, PSUM accum)
    tail: col0 [8, 640] -> PE transpose -> [640, 8] -> PE @ w_t -> S [8, 32]
          ACT: log(S + 1e-30); DVE: * -1/20; DMA out.
General path (continuous mask) adds a mask tensor and one DVE multiply
(e = e * m) before the PE accumulation.
"""

import numpy as np

import concourse.bass as bass
import concourse.tile as tile
from concourse import bacc, mybir

# ---------------------------------------------------------------- constants
N_CORES = 8
BATCH = 64
H_IMG = 480
W_IMG = 640
CROP_START = 192
CROP_H = H_IMG - CROP_START          # 288
NUM_BINS = 32
KAPPA = 20.0

B_PER = BATCH // N_CORES             # 8 images per core
ROWS = B_PER * CROP_H                # 2304
P = 128
NT = ROWS // P                       # 18 row tiles
RPT = P // B_PER                     # 16 rows of each image per tile
WJ = W_IMG // P                      # 5 column chunks of 128

F32 = mybir.dt.float32
F32R = mybir.dt.float32r
U8 = mybir.dt.uint8

USE_F32R = True                      # single-pass PE col-sums (~7e-5 rel err)
# tapered chunk sizes (in 128-row tiles): small ends shorten pipeline
# fill/drain, middle chunks amortize DMA trigger cost
CHUNKS = (1, 1, 2, 2, 2, 2, 2, 2, 2, 2)
assert sum(CHUNKS) == NT
FREE = NT * W_IMG                    # 11520 free elements per partition


class _InitSlim:
    """Skip the Bass-constructor const-AP memsets and the init all-engine
    barrier (~3us of NEFF preamble). Nothing in this kernel reads the
    const APs, and Tile's own preamble/tail barriers provide the sync the
    scheduler relies on."""

    def __enter__(self):
        self._ob = bacc.Bacc.all_engine_barrier
        self._om = bass.BassSharedVectorInterface.memset
        state = {"init_done": False}
        ob, om = self._ob, self._om

        def barrier(s, *a, **k):
            if not state["init_done"]:
                state["init_done"] = True
                return None
            return ob(s, *a, **k)

        def memset(s, ap, c):
            if not state["init_done"] and ap.tensor.name.startswith("const-"):
                return None
            return om(s, ap, c)

        bacc.Bacc.all_engine_barrier = barrier
        bass.BassSharedVectorInterface.memset = memset

    def __exit__(self, *a):
        bacc.Bacc.all_engine_barrier = self._ob
        bass.BassSharedVectorInterface.memset = self._om


def _build_nc(binary_mask: bool) -> bass.Bass:
    with _InitSlim():
        nc = bacc.Bacc(trn_type="TRN2")

    e_dt = F32R if USE_F32R else F32

    depth = nc.dram_tensor("depth", [P, FREE], F32, kind="ExternalInput")
    if not binary_mask:
        maskv = nc.dram_tensor("maskv", [P, FREE], F32, kind="ExternalInput")
    w_t = nc.dram_tensor("w_t", [WJ, P, NUM_BINS], F32, kind="ExternalInput")
    onehot = nc.dram_tensor("onehot", [P, B_PER], F32, kind="ExternalInput")
    ident8 = nc.dram_tensor("ident8", [B_PER, B_PER], F32, kind="ExternalInput")
    out = nc.dram_tensor("out", [B_PER, NUM_BINS], F32, kind="ExternalOutput")

    with tile.TileContext(nc) as tc:
        with (
            tc.tile_pool(name="consts", bufs=1) as consts,
            tc.tile_pool(name="data", bufs=8) as data,
            tc.tile_pool(name="work", bufs=8) as work,
            tc.tile_pool(name="tail", bufs=1) as tail,
            tc.tile_pool(name="psum", bufs=1, space="PSUM") as psum,
        ):
            # constants. oh_buf MUST be the first SBUF allocation and padded
            # to a full 512B row: the f32r LDWEIGHTS path corrupts weight
            # columns 4-7 when the one-hot sits at other offsets/pitches.
            oh_buf = consts.tile([P, P], e_dt)
            oh_sb = oh_buf[:, 0:B_PER]
            nc.scalar.dma_start(
                out=oh_sb,
                in_=onehot[:, :].bitcast(e_dt) if USE_F32R else onehot[:, :],
            )
            wt_sb = consts.tile([P, WJ, NUM_BINS], F32)
            nc.scalar.dma_start(out=wt_sb, in_=w_t.rearrange("j p n -> p j n"))
            id_sb = consts.tile([B_PER, B_PER], F32)
            nc.scalar.dma_start(out=id_sb, in_=ident8[:, :])
            bias_exp = consts.tile([P, 1], F32)
            nc.vector.memset(bias_exp, -KAPPA * 100.0)
            bias_ln = consts.tile([P, 1], F32)
            nc.vector.memset(bias_ln, 1e-30)
            # dummy activation so the Exp table load is hoisted to kernel
            # start, overlapping the first data DMA instead of blocking it
            warm = consts.tile([1, 1], F32)
            nc.scalar.activation(
                warm, bias_ln[0:1, :], mybir.ActivationFunctionType.Exp,
                bias=bias_exp[0:1, :], scale=0.0,
            )

            # stage 1: per-image column sums, accumulated in PSUM.
            col_a = psum.tile([B_PER, 512], F32)
            col_b = psum.tile([B_PER, W_IMG - 512], F32)
            t0 = 0
            for c, ch in enumerate(CHUNKS):
                cw = ch * W_IMG
                off = t0 * W_IMG
                dma_eng = nc.sync
                d_t = data.tile([P, ch, W_IMG], F32, tag=f"d{ch}")
                dma_eng.dma_start(
                    out=d_t,
                    in_=depth[:, off : off + cw].rearrange(
                        "p (n w) -> p n w", w=W_IMG
                    ),
                )
                if binary_mask:
                    e_t = work.tile([P, ch, W_IMG], e_dt, tag=f"e{ch}")
                    nc.scalar.activation(
                        e_t, d_t, mybir.ActivationFunctionType.Exp,
                        bias=bias_exp, scale=-KAPPA,
                    )
                else:
                    m_t = data.tile([P, ch, W_IMG], F32, tag=f"m{ch}")
                    dma_eng.dma_start(
                        out=m_t,
                        in_=maskv[:, off : off + cw].rearrange(
                            "p (n w) -> p n w", w=W_IMG
                        ),
                    )
                    e_f = work.tile([P, ch, W_IMG], F32, tag=f"ef{ch}")
                    nc.scalar.activation(
                        e_f, d_t, mybir.ActivationFunctionType.Exp,
                        bias=bias_exp, scale=-KAPPA,
                    )
                    e_t = work.tile([P, ch, W_IMG], e_dt, tag=f"e{ch}")
                    nc.vector.tensor_mul(e_t, e_f, m_t)

                for n in range(ch):
                    first = t0 + n == 0
                    last = t0 + n == NT - 1
                    nc.tensor.matmul(
                        col_a, oh_sb, e_t[:, n, 0:512],
                        start=first, stop=last,
                    )
                    nc.tensor.matmul(
                        col_b, oh_sb, e_t[:, n, 512:W_IMG],
                        start=first, stop=last,
                    )
                t0 += ch

            # stage 2: S[img, nb] = sum_w col0[img, w] * w_t[w, nb]
            col_sb = tail.tile([B_PER, W_IMG], F32)
            nc.vector.tensor_copy(col_sb[:, 0:512], col_a)
            nc.scalar.copy(col_sb[:, 512:W_IMG], col_b)

            colT_ps = psum.tile([P, WJ, B_PER], F32)
            for j in range(WJ):
                nc.tensor.transpose(
                    colT_ps[:, j, :],
                    col_sb[:, j * P : (j + 1) * P],
                    id_sb,
                )
            colT_sb = tail.tile([P, WJ, B_PER], F32)
            nc.vector.tensor_copy(colT_sb, colT_ps)

            s_ps = psum.tile([B_PER, NUM_BINS], F32)
            for j in range(WJ):
                nc.tensor.matmul(
                    s_ps, colT_sb[:, j, :], wt_sb[:, j, :],
                    start=(j == 0), stop=(j == WJ - 1),
                )

            res = tail.tile([B_PER, NUM_BINS], F32)
            nc.scalar.activation(
                res, s_ps, mybir.ActivationFunctionType.Ln,
                bias=bias_ln[:B_PER], scale=1.0,
            )
            nc.vector.tensor_scalar_mul(res, res, -1.0 / KAPPA)
            nc.sync.dma_start(out=out[:, :], in_=res)

    nc.compile()
    return nc


# ------------------------------------------------------------------ runner
_RUNNERS: dict = {}


def _make_runner(binary_mask: bool):
    """Build the Bass module once and return a persistent jitted SPMD runner.

    Mirrors concourse.bass2jax.run_bass_via_pjrt's multi-core path, but keeps
    the jax.jit object alive across calls so repeat invocations skip
    retracing/recompilation.
    """
    import jax
    from jax.experimental.shard_map import shard_map
    from jax.sharding import Mesh, PartitionSpec

    from concourse import bass2jax, mybir as _mybir

    nc = _build_nc(binary_mask)
    bass2jax.install_neuronx_cc_hook()

    partition_name = (
        nc.partition_id_tensor.name if nc.partition_id_tensor else None
    )
    in_names, out_names, out_avals, zero_outs = [], [], [], []
    for alloc in nc.m.functions[0].allocations:
        if not isinstance(alloc, _mybir.MemoryLocationSet):
            continue
        name = alloc.memorylocations[0].name
        if alloc.kind == "ExternalInput":
            if name != partition_name:
                in_names.append(name)
        elif alloc.kind == "ExternalOutput":
            shape = tuple(alloc.tensor_shape)
            dtype = _mybir.dt.np(alloc.dtype)
            out_names.append(name)
            out_avals.append(jax.core.ShapedArray(shape, dtype))
            zero_outs.append(np.zeros(shape, dtype))
    n_params = len(in_names)
    n_outs = len(out_avals)
    all_in_names = list(in_names) + list(out_names)
    if partition_name is not None:
        all_in_names.append(partition_name)

    def _body(*args):
        operands = list(args)
        if partition_name is not None:
            operands.append(bass2jax.partition_id_tensor())
        outs = bass2jax._bass_exec_p.bind(
            *operands,
            out_avals=tuple(out_avals),
            in_names=tuple(all_in_names),
            out_names=tuple(out_names),
            lowering_input_output_aliases=(),
            sim_require_finite=True,
            sim_require_nnan=True,
            nc=nc,
        )
        return tuple(outs)

    devices = jax.devices()[:N_CORES]
    assert len(devices) == N_CORES, f"need {N_CORES} cores, have {len(devices)}"
    mesh = Mesh(np.asarray(devices), ("core",))
    donate = tuple(range(n_params, n_params + n_outs))
    sharded = jax.jit(
        shard_map(
            _body,
            mesh=mesh,
            in_specs=(PartitionSpec("core"),) * (n_params + n_outs),
            out_specs=(PartitionSpec("core"),) * n_outs,
            check_rep=False,
        ),
        donate_argnums=donate,
        keep_unused=True,
    )

    def run(per_core_in_maps):
        concat_in = [
            np.concatenate([m[name] for m in per_core_in_maps], axis=0)
            for name in in_names
        ]
        concat_zeros = [
            np.zeros((N_CORES * z.shape[0], *z.shape[1:]), z.dtype)
            for z in zero_outs
        ]
        out_arrs = sharded(*concat_in, *concat_zeros)
        return [
            {
                name: np.asarray(out_arrs[i]).reshape(
                    N_CORES, *out_avals[i].shape
                )[c]
                for i, name in enumerate(out_names)
            }
            for c in range(N_CORES)
        ]

    return run


def _get_runner(binary_mask: bool):
    key = "u8" if binary_mask else "f32"
    if key not in _RUNNERS:
        _RUNNERS[key] = _make_runner(binary_mask)
    return _RUNNERS[key]


_ONEHOT = np.zeros((P, B_PER), np.float32)
_ONEHOT[np.arange(P), np.arange(P) // RPT] = 1.0
_IDENT8 = np.eye(B_PER, dtype=np.float32)


def _to_device_layout(a):
    """[N_CORES, B_PER, CROP_H, W] -> [N_CORES, 128, NT*W] so that partition
    p = img*16 + r16 reads its rows (r16::16) contiguously."""
    a = a.reshape(N_CORES, B_PER, NT, RPT, W_IMG)
    a = a.transpose(0, 1, 3, 2, 4)                  # [., img, r16, t, w]
    return np.ascontiguousarray(a.reshape(N_CORES, P, FREE))


def _prep_in_maps(inputs):
    depth_map = np.asarray(inputs["depth_map"], np.float32)
    depth_mask = np.asarray(inputs["depth_mask"], np.float32)
    bin_weights = np.asarray(inputs["bin_weights"], np.float32)

    dc = depth_map[:, CROP_START:, :].reshape(N_CORES, B_PER, CROP_H, W_IMG)
    mcf = depth_mask[:, CROP_START:, :].reshape(N_CORES, B_PER, CROP_H, W_IMG)
    binary = bool(np.all((mcf == 0.0) | (mcf == 1.0)))

    if binary:
        v = np.where(mcf != 0.0, dc - np.float32(100.0), np.float32(0.0))
        v = _to_device_layout(v)
        masks = None
    else:
        v = _to_device_layout(dc - np.float32(100.0))
        masks = _to_device_layout(mcf)

    w_t = np.ascontiguousarray(
        (bin_weights.astype(np.float64) + 1e-10).astype(np.float32).T
    ).reshape(WJ, P, NUM_BINS)

    in_maps = []
    for c in range(N_CORES):
        m = {
            "depth": v[c],
            "w_t": w_t,
            "onehot": _ONEHOT,
            "ident8": _IDENT8,
        }
        if masks is not None:
            m["maskv"] = masks[c]
        in_maps.append(m)
    return in_maps, binary


def kernel(**inputs) -> np.ndarray:
    bin_weights = np.asarray(inputs["bin_weights"], np.float32)
    in_maps, binary = _prep_in_maps(inputs)
    run = _get_runner(binary)
    results = run(in_maps)
    out = np.concatenate([results[c]["out"] for c in range(N_CORES)], axis=0)

    w_sum = bin_weights.sum(axis=-1) * CROP_H
    return np.where(w_sum[None, :] < 1e-6, np.float32(100.0), out).astype(
        np.float32
    )


# revision 36
# speedup vs baseline: 1.0944x; 1.0128x over previous
"""DepthPolarReducer Trainium2 kernel.

Full-input contract: kernel(**inputs) takes the complete arrays and returns the
complete (64, 32) float32 output. The batch is sharded 8 ways across the 8
NeuronCores (pure data parallel, bin_weights replicated, no collectives).

Math (identical to the reference up to f32 rounding):
    dm  = dc*mc + (1-mc)*100                      (cropped rows 192:480)
    out[b, nb] = -log( sum_w (w[nb,w]+1e-10) * sum_h mc*exp(-20*dm) ) / 20
The reference's two-level stable logsumexp collapses algebraically to this
unnormalized form: any term more than ~e^-87 below a bin's dominant column
underflows to 0 in f32 in the reference as well, so results agree to ~1e-4
for any random-fill data (verified against the oracle).

Host prep (cheap affine/masking/layout only — all transcendentals and
reductions run on device):
  - crop rows, shard batch, subtract 100, apply binary mask (masked -> 0,
    so exp(-20*0 - 2000) == 0 on device), and lay rows out so each DMA is
    per-partition contiguous and every 128-row tile holds 16 rows of each
    of the core's 8 images (-> constant one-hot PE reduction matrix).

Device schedule per core (fast path, binary mask):
    6 chunks of [128, 3*640]:
        ACT: e = exp(-20*v - 2000)        (v = masked depth-100 from host)
        PE:  col0[img, :] += onehot.T @ e   (f32r single-pass, PSUM accum)
    tail: col0 [8, 640] -> PE transpose -> [640, 8] -> PE @ w_t -> S [8, 32]
          ACT: log(S + 1e-30); DVE: * -1/20; DMA out.
General path (continuous mask) adds a mask tensor and one DVE multiply
(e = e * m) before the PE accumulation.
"""

import numpy as np

import concourse.bass as bass
import concourse.tile as tile
from concourse import bacc, mybir

# ---------------------------------------------------------------- constants
N_CORES = 8
BATCH = 64
H_IMG = 480
W_IMG = 640
CROP_START = 192
CROP_H = H_IMG - CROP_START          # 288
NUM_BINS = 32
KAPPA = 20.0

B_PER = BATCH // N_CORES             # 8 images per core
ROWS = B_PER * CROP_H                # 2304
P = 128
NT = ROWS // P                       # 18 row tiles
RPT = P // B_PER                     # 16 rows of each image per tile
WJ = W_IMG // P                      # 5 column chunks of 128

F32 = mybir.dt.float32
F32R = mybir.dt.float32r
U8 = mybir.dt.uint8

USE_F32R = True                      # single-pass PE col-sums (~7e-5 rel err)
# tapered chunk sizes (in 128-row tiles): small ends shorten pipeline
# fill/drain, middle chunks amortize DMA trigger cost
CHUNKS = (1, 1, 2, 2, 2, 2, 2, 2, 2, 2)
assert sum(CHUNKS) == NT
FREE = NT * W_IMG                    # 11520 free elements per partition


class _InitSlim:
    """Skip the Bass-constructor const-AP memsets and the init all-engine
    barrier (~3us of NEFF preamble). Nothing in this kernel reads the
    const APs, and Tile's own preamble/tail barriers provide the sync the
    scheduler relies on."""

    def __enter__(self):
        self._ob = bacc.Bacc.all_engine_barrier
        self._om = bass.BassSharedVectorInterface.memset
        state = {"init_done": False}
        ob, om = self._ob, self._om

        def barrier(s, *a, **k):
            if not state["init_done"]:
                state["init_done"] = True
                return None
            return ob(s, *a, **k)

        def memset(s, ap, c):
            if not state["init_done"] and ap.tensor.name.startswith("const-"):
                return None
            return om(s, ap, c)

        bacc.Bacc.all_engine_barrier = barrier
        bass.BassSharedVectorInterface.memset = memset

    def __exit__(self, *a):
        bacc.Bacc.all_engine_barrier = self._ob
        bass.BassSharedVectorInterface.memset = self._om


def _build_nc() -> bass.Bass:
    with _InitSlim():
        nc = bacc.Bacc(trn_type="TRN2")

    e_dt = F32R if USE_F32R else F32

    depth = nc.dram_tensor("depth", [P, FREE], F32, kind="ExternalInput")
    w_t = nc.dram_tensor("w_t", [WJ, P, NUM_BINS], F32, kind="ExternalInput")
    onehot = nc.dram_tensor("onehot", [P, B_PER], F32, kind="ExternalInput")
    ident8 = nc.dram_tensor("ident8", [B_PER, B_PER], F32, kind="ExternalInput")
    out = nc.dram_tensor("out", [B_PER, NUM_BINS], F32, kind="ExternalOutput")

    with tile.TileContext(nc) as tc:
        with (
            tc.tile_pool(name="consts", bufs=1) as consts,
            tc.tile_pool(name="data", bufs=8) as data,
            tc.tile_pool(name="work", bufs=8) as work,
            tc.tile_pool(name="tail", bufs=1) as tail,
            tc.tile_pool(name="psum", bufs=1, space="PSUM") as psum,
        ):
            # constants. oh_buf MUST be the first SBUF allocation and padded
            # to a full 512B row: the f32r LDWEIGHTS path corrupts weight
            # columns 4-7 when the one-hot sits at other offsets/pitches.
            oh_buf = consts.tile([P, P], e_dt)
            oh_sb = oh_buf[:, 0:B_PER]
            nc.scalar.dma_start(
                out=oh_sb,
                in_=onehot[:, :].bitcast(e_dt) if USE_F32R else onehot[:, :],
            )
            wt_sb = consts.tile([P, WJ, NUM_BINS], F32)
            nc.scalar.dma_start(out=wt_sb, in_=w_t.rearrange("j p n -> p j n"))
            id_sb = consts.tile([B_PER, B_PER], F32)
            nc.scalar.dma_start(out=id_sb, in_=ident8[:, :])
            bias_exp = consts.tile([P, 1], F32)
            nc.vector.memset(bias_exp, -KAPPA * 100.0)
            bias_ln = consts.tile([P, 1], F32)
            nc.vector.memset(bias_ln, 1e-30)
            # dummy activation so the Exp table load is hoisted to kernel
            # start, overlapping the first data DMA instead of blocking it
            warm = consts.tile([1, 1], F32)
            nc.scalar.activation(
                warm, bias_ln[0:1, :], mybir.ActivationFunctionType.Exp,
                bias=bias_exp[0:1, :], scale=0.0,
            )

            # stage 1: per-image column sums, accumulated in PSUM.
            col_a = psum.tile([B_PER, 512], F32)
            col_b = psum.tile([B_PER, W_IMG - 512], F32)
            t0 = 0
            for c, ch in enumerate(CHUNKS):
                cw = ch * W_IMG
                off = t0 * W_IMG
                dma_eng = nc.sync
                d_t = data.tile([P, ch, W_IMG], F32, tag="d")
                dma_eng.dma_start(
                    out=d_t,
                    in_=depth[:, off : off + cw].rearrange(
                        "p (n w) -> p n w", w=W_IMG
                    ),
                )
                e_t = work.tile([P, ch, W_IMG], e_dt, tag="e")
                nc.scalar.activation(
                    e_t, d_t, mybir.ActivationFunctionType.Exp,
                    bias=bias_exp, scale=-KAPPA,
                )

                for n in range(ch):
                    first = t0 + n == 0
                    last = t0 + n == NT - 1
                    nc.tensor.matmul(
                        col_a, oh_sb, e_t[:, n, 0:512],
                        start=first, stop=last,
                    )
                    nc.tensor.matmul(
                        col_b, oh_sb, e_t[:, n, 512:W_IMG],
                        start=first, stop=last,
                    )
                t0 += ch

            # stage 2: S[img, nb] = sum_w col0[img, w] * w_t[w, nb]
            col_sb = tail.tile([B_PER, W_IMG], F32)
            nc.vector.tensor_copy(col_sb[:, 0:512], col_a)
            nc.scalar.copy(col_sb[:, 512:W_IMG], col_b)

            colT_ps = psum.tile([P, WJ, B_PER], F32)
            for j in range(WJ):
                nc.tensor.transpose(
                    colT_ps[:, j, :],
                    col_sb[:, j * P : (j + 1) * P],
                    id_sb,
                )
            colT_sb = tail.tile([P, WJ, B_PER], F32)
            nc.vector.tensor_copy(colT_sb, colT_ps)

            s_ps = psum.tile([B_PER, NUM_BINS], F32)
            for j in range(WJ):
                nc.tensor.matmul(
                    s_ps, colT_sb[:, j, :], wt_sb[:, j, :],
                    start=(j == 0), stop=(j == WJ - 1),
                )

            res = tail.tile([B_PER, NUM_BINS], F32)
            nc.scalar.activation(
                res, s_ps, mybir.ActivationFunctionType.Ln,
                bias=bias_ln[:B_PER], scale=1.0,
            )
            nc.vector.tensor_scalar_mul(res, res, -1.0 / KAPPA)
            nc.sync.dma_start(out=out[:, :], in_=res)

    nc.compile()
    return nc


# ------------------------------------------------------------------ runner
_RUNNERS: dict = {}


def _make_runner():
    """Build the Bass module once and return a persistent jitted SPMD runner.

    Mirrors concourse.bass2jax.run_bass_via_pjrt's multi-core path, but keeps
    the jax.jit object alive across calls so repeat invocations skip
    retracing/recompilation.
    """
    import jax
    from jax.experimental.shard_map import shard_map
    from jax.sharding import Mesh, PartitionSpec

    from concourse import bass2jax, mybir as _mybir

    nc = _build_nc()
    bass2jax.install_neuronx_cc_hook()

    partition_name = (
        nc.partition_id_tensor.name if nc.partition_id_tensor else None
    )
    in_names, out_names, out_avals, zero_outs = [], [], [], []
    for alloc in nc.m.functions[0].allocations:
        if not isinstance(alloc, _mybir.MemoryLocationSet):
            continue
        name = alloc.memorylocations[0].name
        if alloc.kind == "ExternalInput":
            if name != partition_name:
                in_names.append(name)
        elif alloc.kind == "ExternalOutput":
            shape = tuple(alloc.tensor_shape)
            dtype = _mybir.dt.np(alloc.dtype)
            out_names.append(name)
            out_avals.append(jax.core.ShapedArray(shape, dtype))
            zero_outs.append(np.zeros(shape, dtype))
    n_params = len(in_names)
    n_outs = len(out_avals)
    all_in_names = list(in_names) + list(out_names)
    if partition_name is not None:
        all_in_names.append(partition_name)

    def _body(*args):
        operands = list(args)
        if partition_name is not None:
            operands.append(bass2jax.partition_id_tensor())
        outs = bass2jax._bass_exec_p.bind(
            *operands,
            out_avals=tuple(out_avals),
            in_names=tuple(all_in_names),
            out_names=tuple(out_names),
            lowering_input_output_aliases=(),
            sim_require_finite=True,
            sim_require_nnan=True,
            nc=nc,
        )
        return tuple(outs)

    devices = jax.devices()[:N_CORES]
    assert len(devices) == N_CORES, f"need {N_CORES} cores, have {len(devices)}"
    mesh = Mesh(np.asarray(devices), ("core",))
    donate = tuple(range(n_params, n_params + n_outs))
    sharded = jax.jit(
        shard_map(
            _body,
            mesh=mesh,
            in_specs=(PartitionSpec("core"),) * (n_params + n_outs),
            out_specs=(PartitionSpec("core"),) * n_outs,
            check_rep=False,
        ),
        donate_argnums=donate,
        keep_unused=True,
    )

    def run(per_core_in_maps):
        concat_in = [
            np.concatenate([m[name] for m in per_core_in_maps], axis=0)
            for name in in_names
        ]
        concat_zeros = [
            np.zeros((N_CORES * z.shape[0], *z.shape[1:]), z.dtype)
            for z in zero_outs
        ]
        out_arrs = sharded(*concat_in, *concat_zeros)
        return [
            {
                name: np.asarray(out_arrs[i]).reshape(
                    N_CORES, *out_avals[i].shape
                )[c]
                for i, name in enumerate(out_names)
            }
            for c in range(N_CORES)
        ]

    return run


def _get_runner():
    if "k" not in _RUNNERS:
        _RUNNERS["k"] = _make_runner()
    return _RUNNERS["k"]


_ONEHOT = np.zeros((P, B_PER), np.float32)
_ONEHOT[np.arange(P), np.arange(P) // RPT] = 1.0
_IDENT8 = np.eye(B_PER, dtype=np.float32)


def _to_device_layout(a):
    """[N_CORES, B_PER, CROP_H, W] -> [N_CORES, 128, NT*W] so that partition
    p = img*16 + r16 reads its rows (r16::16) contiguously."""
    a = a.reshape(N_CORES, B_PER, NT, RPT, W_IMG)
    a = a.transpose(0, 1, 3, 2, 4)                  # [., img, r16, t, w]
    return np.ascontiguousarray(a.reshape(N_CORES, P, FREE))


def _prep_in_maps(inputs):
    """Host prep: crop, shard, fold the mask into the exp argument, reorder
    rows to the device layout. The device computes exp(-20*v - 2000) and
    reduces; so v must satisfy  exp(-20*v - 2000) == mc * exp(-20*dm):
      - binary mask:    v = d - 100 where mc==1, else 0 (exp(-2000) == 0)
      - continuous:     v = mc*(d-100) - ln(mc)/20, masked pixels -> 1e4
    """
    depth_map = np.asarray(inputs["depth_map"], np.float32)
    depth_mask = np.asarray(inputs["depth_mask"], np.float32)
    bin_weights = np.asarray(inputs["bin_weights"], np.float32)

    dc = depth_map[:, CROP_START:, :].reshape(N_CORES, B_PER, CROP_H, W_IMG)
    mcf = depth_mask[:, CROP_START:, :].reshape(N_CORES, B_PER, CROP_H, W_IMG)
    binary = bool(np.all((mcf == 0.0) | (mcf == 1.0)))

    if binary:
        v = np.where(mcf != 0.0, dc - np.float32(100.0), np.float32(0.0))
    else:
        with np.errstate(divide="ignore", invalid="ignore"):
            v = mcf * (dc - np.float32(100.0)) - np.log(mcf) / np.float32(
                KAPPA
            )
        v = np.where(mcf == 0.0, np.float32(1e4), v).astype(np.float32)
    v = _to_device_layout(v)

    w_t = np.ascontiguousarray(
        (bin_weights.astype(np.float64) + 1e-10).astype(np.float32).T
    ).reshape(WJ, P, NUM_BINS)

    in_maps = [
        {"depth": v[c], "w_t": w_t, "onehot": _ONEHOT, "ident8": _IDENT8}
        for c in range(N_CORES)
    ]
    return in_maps, binary


def kernel(**inputs) -> np.ndarray:
    bin_weights = np.asarray(inputs["bin_weights"], np.float32)
    in_maps, _ = _prep_in_maps(inputs)
    run = _get_runner()
    results = run(in_maps)
    out = np.concatenate([results[c]["out"] for c in range(N_CORES)], axis=0)

    w_sum = bin_weights.sum(axis=-1) * CROP_H
    return np.where(w_sum[None, :] < 1e-6, np.float32(100.0), out).astype(
        np.float32
    )
